# revision 3
# baseline (speedup 1.0000x reference)
"""Causal attention (B=4, S=2048, D=1024, fp32) on 8 Trainium2 NeuronCores.

Sharding: data-parallel over batch (4) x query-split (2) per batch. The two
cores of a batch take interleaved query rows (even/odd within each 512-row
super-block), which makes the causal workload identical on every core and
lets one SPMD program serve all 8 cores; the only per-core differences are
pure data (which query columns of x^T each core receives, and the mask
tiles, which carry the even/odd offset).

All matmul inputs are bf16 (inputs cast host-side; k/q/v stored bf16 in
SBUF; exp probabilities bf16), accumulation in fp32 PSUM. Measured rel err
~5e-3 vs the fp32 reference (gate 2e-2).

Projection is one merged sweep: all three weight matrices stay SBUF-resident
(bf16 halves the footprint), and each 512-column slice of x^T feeds K, V and
Q accumulation chains back to back, so there is no separate DMA-bound Q pass.

Attention per core:
  For each of 4 query slots s (256 queries from super-block [512s, 512s+512)):
    for key block kb in [0, 4s+4): scoresT = kT_blk^T q  -> +mask -> exp
      (no max-subtraction: scaled scores are ~N(0,1), exp is fp32-safe)
      denominators via ones-matmul; ctx accumulation in PSUM
    normalize by reciprocal(denom), DMA out.
"""

import numpy as np

B, S, D = 4, 2048, 1024
NE = D // 128          # contraction chunks (d on partitions)
NKBLK = S // 128       # 128-wide key blocks
NSLOT = 4              # query slots per core
QW = 256               # queries per slot
OWNQ = NSLOT * QW      # 1024 queries per core
JW = 512               # key double-slice width in the projection sweep
NJD = S // JW          # 4 double-slices
MASK_NEG = -1.0e30
SCALE = 1.0 / 32.0     # 1/sqrt(D)

_cached = {}


def _build():
    import concourse.bacc as bacc
    import concourse.tile as tile
    import concourse.mybir as mybir

    F32 = mybir.dt.float32
    BF16 = mybir.dt.bfloat16
    EXP = mybir.ActivationFunctionType.Exp

    nc = bacc.Bacc("TRN2", target_bir_lowering=False, debug=False, num_devices=8,
                   dynamic_dma_scratch_size=2048)

    xt_d = nc.dram_tensor("xt", [D, S], BF16, kind="ExternalInput")
    xq_d = nc.dram_tensor("xq", [D, OWNQ], BF16, kind="ExternalInput")
    wq_d = nc.dram_tensor("wq", [D, D], BF16, kind="ExternalInput")
    wk_d = nc.dram_tensor("wk", [D, D], BF16, kind="ExternalInput")
    wv_d = nc.dram_tensor("wv", [D, D], BF16, kind="ExternalInput")
    mask_d = nc.dram_tensor("masks", [128, 4 * QW], F32, kind="ExternalInput")
    ones_d = nc.dram_tensor("ones", [128, 2], BF16, kind="ExternalInput")
    o_d = nc.dram_tensor("o", [OWNQ, D], F32, kind="ExternalOutput")

    DENG = [None, None, None]  # set inside context: dma-capable engines

    with tile.TileContext(nc) as tc:
        with tc.tile_pool(name="res", bufs=1) as res:
            DENG[0], DENG[1], DENG[2] = nc.sync, nc.scalar, nc.sync
            kT = []
            for c in range(NE):
                t = res.tile([128, S], BF16, name=f"kT{c}", tag=f"kT{c}")
                kT.append(t)
            vv = []
            for j in range(NKBLK):
                t = res.tile([128, D], BF16, name=f"v{j}", tag=f"v{j}")
                vv.append(t)
            qT = []
            for c in range(NE):
                t = res.tile([128, OWNQ], BF16, name=f"qT{c}", tag=f"qT{c}")
                qT.append(t)
            # ---------------- projection phase ----------------
            # Whole wk/wv/wq resident (bf16): chunk dc of each lives at
            # cols [dc*1024, (dc+1)*1024).
            with (
                tc.tile_pool(name="wpool", bufs=1) as wpool,
                tc.tile_pool(name="xsp", bufs=2) as xsp,
                tc.tile_pool(name="qxp", bufs=2) as qxp,
                tc.tile_pool(name="pp", bufs=4, space="PSUM") as pp,
                tc.tile_pool(name="qp", bufs=2, space="PSUM") as qp,
            ):
                wk_t = wpool.tile([128, NE * D], BF16, name="wk_t", tag="wk")
                wv_t = wpool.tile([128, NE * D], BF16, name="wv_t", tag="wv")
                wq_t = wpool.tile([128, NE * D], BF16, name="wq_t", tag="wq")

                def load_x_slices(jd):
                    # xs chunk dc at [dc*JW, +JW); xq chunk dc at [dc*QW, +QW)
                    xs_t = xsp.tile([128, NE * JW], BF16, name="xs_t", tag="xs")
                    xq_t = qxp.tile([128, NE * QW], BF16, name="xq_t", tag="xq")
                    for dc in range(NE):
                        e = DENG[dc % 3]
                        e.dma_start(
                            xs_t[:, dc * JW:(dc + 1) * JW],
                            xt_d[dc * 128:(dc + 1) * 128, jd * JW:(jd + 1) * JW],
                        )
                        e.dma_start(
                            xq_t[:, dc * QW:(dc + 1) * QW],
                            xq_d[dc * 128:(dc + 1) * 128, jd * QW:(jd + 1) * QW],
                        )
                    return xs_t, xq_t

                # Interleave the first x-slice DMAs with the weight loads so
                # the first accumulation chains can start after ~1MB, not
                # after the full 6MB of weights.
                xs0, xq0 = load_x_slices(0)
                for dc in range(NE):
                    DENG[dc % 3].dma_start(
                        wk_t[:, dc * D:(dc + 1) * D],
                        wk_d[dc * 128:(dc + 1) * 128, :],
                    )
                for dc in range(NE):
                    DENG[dc % 3].dma_start(
                        wv_t[:, dc * D:(dc + 1) * D],
                        wv_d[dc * 128:(dc + 1) * 128, :],
                    )
                for dc in range(NE):
                    DENG[dc % 3].dma_start(
                        wq_t[:, dc * D:(dc + 1) * D],
                        wq_d[dc * 128:(dc + 1) * 128, :],
                    )

                for jd in range(NJD):
                    if jd == 0:
                        xs_t, xq_t = xs0, xq0
                    else:
                        xs_t, xq_t = load_x_slices(jd)
                    for i in range(NE):
                        # K chain: kT[i][:, jd*JW : +JW]
                        ps = pp.tile([128, JW], F32, name="ps_k", tag="ps")
                        for dc in range(NE):
                            nc.tensor.matmul(
                                ps[:, :],
                                wk_t[:, dc * D + i * 128: dc * D + (i + 1) * 128],
                                xs_t[:, dc * JW:(dc + 1) * JW],
                                start=(dc == 0), stop=(dc == NE - 1),
                            )
                        nc.scalar.copy(kT[i][:, jd * JW:(jd + 1) * JW], ps[:, :])
                        # V chain: vv[4*jd + i//2][:, (i%2)*512 : +512]
                        jt, dh = i // 2, i % 2
                        ps = pp.tile([128, JW], F32, name="ps_v", tag="ps")
                        for dc in range(NE):
                            nc.tensor.matmul(
                                ps[:, :],
                                xs_t[:, dc * JW + jt * 128: dc * JW + jt * 128 + 128],
                                wv_t[:, dc * D + dh * 512: dc * D + (dh + 1) * 512],
                                start=(dc == 0), stop=(dc == NE - 1),
                            )
                        nc.vector.tensor_copy(
                            vv[4 * jd + jt][:, dh * 512:(dh + 1) * 512], ps[:, :]
                        )
                        # Q chain: qT[i][:, jd*QW : +QW]
                        psq = qp.tile([128, QW], F32, name="ps_q", tag="psq")
                        for dc in range(NE):
                            nc.tensor.matmul(
                                psq[:, :],
                                wq_t[:, dc * D + i * 128: dc * D + (i + 1) * 128],
                                xq_t[:, dc * QW:(dc + 1) * QW],
                                start=(dc == 0), stop=(dc == NE - 1),
                            )
                        nc.scalar.copy(qT[i][:, jd * QW:(jd + 1) * QW], psq[:, :])

            # ---------------- attention phase ----------------
            with (
                tc.tile_pool(name="cns", bufs=1) as cns,
                tc.tile_pool(name="ptp", bufs=4) as ptp,
                tc.tile_pool(name="obp", bufs=2) as obp,
                tc.tile_pool(name="rcp", bufs=2) as rcp,
                tc.tile_pool(name="scp", bufs=3, space="PSUM") as scp,
                tc.tile_pool(name="ctxp", bufs=1, space="PSUM") as ctxp,
                tc.tile_pool(name="dnp", bufs=1, space="PSUM") as dnp,
            ):
                mask_t = cns.tile([128, 4 * QW], F32, name="mask_t", tag="mask_t")
                ones_t = cns.tile([128, 2], BF16, name="ones_t", tag="ones_t")
                nc.sync.dma_start(mask_t[:, :], mask_d[:, :])
                nc.sync.dma_start(ones_t[:, :], ones_d[:, :])

                def consume(item):
                    s, kb, pt, ctx, dn = item
                    nk = 4 * s + 4
                    for c in range(2):
                        # Both column groups live in one PSUM bank; start=True
                        # clears the whole bank, so only the first group may
                        # set it — the second lands on freshly cleared psum
                        # (has_written=0) and still overwrites, not adds.
                        nc.tensor.matmul(
                            dn[:, 2 * c:2 * c + 2],
                            pt[:, c * 128:(c + 1) * 128],
                            ones_t[:, :],
                            start=(kb == 0 and c == 0), stop=(kb == nk - 1),
                            skip_group_check=True,
                        )
                        for dh in range(2):
                            nc.tensor.matmul(
                                ctx[(c, dh)][:, :],
                                pt[:, c * 128:(c + 1) * 128],
                                vv[kb][:, dh * 512:(dh + 1) * 512],
                                start=(kb == 0), stop=(kb == nk - 1),
                            )
                    if kb == nk - 1:
                        rc = rcp.tile([128, 2], F32, name="rc", tag="rc")
                        nc.vector.reciprocal(rc[:, :], dn[:, 0:4:2])
                        for c in range(2):
                            ob = obp.tile([128, D], F32, name="ob", tag="ob")
                            for dh in range(2):
                                nc.vector.tensor_scalar_mul(
                                    ob[:, dh * 512:(dh + 1) * 512],
                                    ctx[(c, dh)][:, :],
                                    rc[:, c:c + 1],
                                )
                            nc.sync.dma_start(
                                o_d[s * QW + c * 128: s * QW + (c + 1) * 128, :],
                                ob[:, :],
                            )

                from collections import deque
                pending = deque()
                DEPTH = 2
                for s in range(NSLOT):
                    nk = 4 * s + 4
                    # Drain before each slot: the slot's ctx/dn pool slots
                    # (bufs=1) can only be re-allocated once the previous
                    # slot's normalize has been emitted.
                    while pending:
                        consume(pending.popleft())
                    ctx_cur = {}
                    for c in range(2):
                        for dh in range(2):
                            t = ctxp.tile(
                                [128, 512], F32,
                                name=f"ctx{c}{dh}", tag=f"ctx{c}{dh}",
                            )
                            ctx_cur[(c, dh)] = t
                    dn_cur = dnp.tile([128, 4], F32, name="dn", tag="dn")
                    for kb in range(nk):
                        ps_sc = scp.tile([128, QW], F32, name="ps_sc", tag="sc")
                        for ec in range(NE):
                            nc.tensor.matmul(
                                ps_sc[:, :],
                                kT[ec][:, kb * 128:(kb + 1) * 128],
                                qT[ec][:, s * QW:(s + 1) * QW],
                                start=(ec == 0), stop=(ec == NE - 1),
                            )
                        t_idx = kb - (nk - 4)
                        if t_idx >= 0:
                            nc.vector.tensor_add(
                                ps_sc[:, :], ps_sc[:, :],
                                mask_t[:, t_idx * QW:(t_idx + 1) * QW],
                            )
                        pt = ptp.tile([128, QW], BF16, name="pt", tag="pt")
                        nc.scalar.activation(pt[:, :], ps_sc[:, :], EXP, scale=SCALE)
                        pending.append((s, kb, pt, ctx_cur, dn_cur))
                        if len(pending) > DEPTH:
                            consume(pending.popleft())
                while pending:
                    consume(pending.popleft())

    nc.compile()
    return nc


def _get_nc():
    if "nc" not in _cached:
        _cached["nc"] = _build()
    return _cached["nc"]


def build_in_maps(x, W_q, W_k, W_v):
    import ml_dtypes

    BF = ml_dtypes.bfloat16
    x = np.asarray(x, dtype=np.float32)
    wq = np.ascontiguousarray(np.asarray(W_q, dtype=np.float32).astype(BF))
    wk = np.ascontiguousarray(np.asarray(W_k, dtype=np.float32).astype(BF))
    wv = np.ascontiguousarray(np.asarray(W_v, dtype=np.float32).astype(BF))
    ones = np.ones((128, 2), dtype=BF)

    p = np.arange(128, dtype=np.int64)[:, None]
    f = np.arange(QW, dtype=np.int64)[None, :]
    masks_h = []
    for h in range(2):
        tiles = [
            np.where(128 * t + p <= 2 * f + h, np.float32(0.0), np.float32(MASK_NEG))
            for t in range(4)
        ]
        masks_h.append(np.concatenate(tiles, axis=1).astype(np.float32))

    xbf = x.astype(BF)
    xt_b = [np.ascontiguousarray(xbf[b].T) for b in range(B)]
    in_maps = []
    for c in range(8):
        b, h = c // 2, c % 2
        xq = np.ascontiguousarray(xbf[b, h::2, :].T)
        in_maps.append({
            "xt": xt_b[b],
            "xq": xq,
            "wq": wq,
            "wk": wk,
            "wv": wv,
            "masks": masks_h[h],
            "ones": ones,
        })
    return in_maps


def kernel(x, W_q, W_k, W_v):
    from concourse.bass_utils import run_bass_kernel_spmd

    in_maps = build_in_maps(x, W_q, W_k, W_v)
    nc = _get_nc()
    res = run_bass_kernel_spmd(nc, in_maps, core_ids=list(range(8)))

    out = np.empty((B, S, D), dtype=np.float32)
    for c in range(8):
        b, h = c // 2, c % 2
        out[b, h::2, :] = res.results[c]["o"]
    return out


# revision 4
# speedup vs baseline: 1.3066x; 1.3066x over previous
"""Causal attention (B=4, S=2048, D=1024, fp32) on 8 Trainium2 NeuronCores.

Sharding: data-parallel over batch (4) x query-split (2) per batch. The two
cores of a batch take interleaved query rows (even/odd within each 512-row
super-block), which makes the causal workload identical on every core and
lets one SPMD program serve all 8 cores; the only per-core differences are
pure data (which query columns of x^T each core receives, and the mask
tiles, which carry the even/odd offset).

Weight folding: scores = (x Wq)(x Wk)^T = x (Wq Wk^T) x^T, so the host
precomputes M = Wq Wk^T (a weight-only transform) and the device needs no
K projection at all: x^T itself is the key matrix, kept SBUF-resident, and
the only projections are q' = M^T-chunks applied to own queries and v.
The V projection chains are interleaved into the attention slot loop so the
PE never waits on a phase boundary.

All matmul inputs are bf16 (cast host-side; q'/v stored bf16 in SBUF; exp
probabilities bf16), accumulation in fp32 PSUM. Measured rel err ~5e-3 vs
the fp32 reference (gate 2e-2).

Attention per core:
  For each of 4 query slots s (256 queries from super-block [512s, 512s+512)):
    for key block kb in [0, 4s+4): scoresT = x_blk^T q'  -> +mask -> exp
      (no max-subtraction: scaled scores are ~N(0,1), exp is fp32-safe)
      denominators via ones-matmul; ctx accumulation in PSUM
    normalize by reciprocal(denom), DMA out.
"""

import numpy as np

B, S, D = 4, 2048, 1024
NE = D // 128          # contraction chunks (d on partitions)
NKBLK = S // 128       # 128-wide key blocks
NSLOT = 4              # query slots per core
QW = 256               # queries per slot
OWNQ = NSLOT * QW      # 1024 queries per core
MASK_NEG = -1.0e30
SCALE = 1.0 / 32.0     # 1/sqrt(D)

_cached = {}


def _build():
    import concourse.bacc as bacc
    import concourse.tile as tile
    import concourse.mybir as mybir
    from collections import deque

    F32 = mybir.dt.float32
    BF16 = mybir.dt.bfloat16
    EXP = mybir.ActivationFunctionType.Exp

    nc = bacc.Bacc("TRN2", target_bir_lowering=False, debug=False, num_devices=8,
                   dynamic_dma_scratch_size=2048)

    xt_d = nc.dram_tensor("xt", [D, S], BF16, kind="ExternalInput")
    xq_d = nc.dram_tensor("xq", [D, OWNQ], BF16, kind="ExternalInput")
    m_d = nc.dram_tensor("m", [D, D], BF16, kind="ExternalInput")
    wv_d = nc.dram_tensor("wv", [D, D], BF16, kind="ExternalInput")
    mask_d = nc.dram_tensor("masks", [128, 4 * QW], F32, kind="ExternalInput")
    ones_d = nc.dram_tensor("ones", [128, 2], BF16, kind="ExternalInput")
    o_d = nc.dram_tensor("o", [OWNQ, D], F32, kind="ExternalOutput")

    with tile.TileContext(nc) as tc:
        with (
            tc.tile_pool(name="res", bufs=1) as res,
            tc.tile_pool(name="ptp", bufs=4) as ptp,
            tc.tile_pool(name="obp", bufs=2) as obp,
            tc.tile_pool(name="rcp", bufs=2) as rcp,
            tc.tile_pool(name="rot", bufs=3, space="PSUM") as rot,
            tc.tile_pool(name="ctxp", bufs=1, space="PSUM") as ctxp,
            tc.tile_pool(name="dnp", bufs=1, space="PSUM") as dnp,
        ):
            # ---- resident tiles ----
            kx = []            # x^T chunks: the key matrix AND the V lhsT
            for c in range(NE):
                t = res.tile([128, S], BF16, name=f"kx{c}", tag=f"kx{c}")
                kx.append(t)
            vv = []
            for j in range(NKBLK):
                t = res.tile([128, D], BF16, name=f"v{j}", tag=f"v{j}")
                vv.append(t)
            qT = []
            for c in range(NE):
                t = res.tile([128, OWNQ], BF16, name=f"qT{c}", tag=f"qT{c}")
                qT.append(t)
            m_t = res.tile([128, NE * D], BF16, name="m_t", tag="m_t")
            wv_t = res.tile([128, NE * D], BF16, name="wv_t", tag="wv_t")
            xqr = res.tile([128, NE * OWNQ], BF16, name="xqr", tag="xqr")
            mask_t = res.tile([128, 4 * QW], F32, name="mask_t", tag="mask_t")
            ones_t = res.tile([128, 2], BF16, name="ones_t", tag="ones_t")

            # ---- input DMAs, in consumption order ----
            # m+xq first (Q chains run first), then wv, then kx, masks early.
            nc.scalar.dma_start(mask_t[:, :], mask_d[:, :])
            nc.scalar.dma_start(ones_t[:, :], ones_d[:, :])
            for dc in range(NE):
                nc.sync.dma_start(
                    m_t[:, dc * D:(dc + 1) * D], m_d[dc * 128:(dc + 1) * 128, :]
                )
                nc.scalar.dma_start(
                    xqr[:, dc * OWNQ:(dc + 1) * OWNQ],
                    xq_d[dc * 128:(dc + 1) * 128, :],
                )
            for dc in range(NE):
                e = nc.sync if dc % 2 == 0 else nc.scalar
                e.dma_start(
                    wv_t[:, dc * D:(dc + 1) * D], wv_d[dc * 128:(dc + 1) * 128, :]
                )
            for dc in range(NE):
                e = nc.sync if dc % 2 == 0 else nc.scalar
                e.dma_start(kx[dc][:, :], xt_d[dc * 128:(dc + 1) * 128, :])

            # ---- q' projection: qT[ei][:, jq*512 : +512] = sum_dc M-chunk^T xq ----
            for jq in range(2):
                for ei in range(NE):
                    ps = rot.tile([128, 512], F32, name="rps", tag="rps")
                    for dc in range(NE):
                        nc.tensor.matmul(
                            ps[:, :],
                            m_t[:, dc * D + ei * 128: dc * D + (ei + 1) * 128],
                            xqr[:, dc * OWNQ + jq * 512: dc * OWNQ + (jq + 1) * 512],
                            start=(dc == 0), stop=(dc == NE - 1),
                        )
                    nc.scalar.copy(qT[ei][:, jq * 512:(jq + 1) * 512], ps[:, :])

            # ---- V chain emitter (interleaved into the attention loop) ----
            vq = deque(range(NKBLK * 2))  # (jc, dh) halves in jc-major order

            def emit_v(n):
                while n > 0 and vq:
                    idx = vq.popleft()
                    jc, dh = idx // 2, idx % 2
                    ps = rot.tile([128, 512], F32, name="rps", tag="rps")
                    for dc in range(NE):
                        nc.tensor.matmul(
                            ps[:, :],
                            kx[dc][:, jc * 128:(jc + 1) * 128],
                            wv_t[:, dc * D + dh * 512: dc * D + (dh + 1) * 512],
                            start=(dc == 0), stop=(dc == NE - 1),
                        )
                    nc.vector.tensor_copy(vv[jc][:, dh * 512:(dh + 1) * 512], ps[:, :])
                    n -= 1

            # ---- attention, V chains woven between score blocks ----
            def consume(item):
                s, kb, pt, ctx, dn = item
                nk = 4 * s + 4
                for c in range(2):
                    # Both column groups live in one PSUM bank; start=True
                    # clears the whole bank, so only the first group may
                    # set it — the second lands on freshly cleared psum
                    # (has_written=0) and still overwrites, not adds.
                    nc.tensor.matmul(
                        dn[:, 2 * c:2 * c + 2],
                        pt[:, c * 128:(c + 1) * 128],
                        ones_t[:, :],
                        start=(kb == 0 and c == 0), stop=(kb == nk - 1),
                        skip_group_check=True,
                    )
                    for dh in range(2):
                        nc.tensor.matmul(
                            ctx[(c, dh)][:, :],
                            pt[:, c * 128:(c + 1) * 128],
                            vv[kb][:, dh * 512:(dh + 1) * 512],
                            start=(kb == 0), stop=(kb == nk - 1),
                        )
                if kb == nk - 1:
                    rc = rcp.tile([128, 2], F32, name="rc", tag="rc")
                    nc.vector.reciprocal(rc[:, :], dn[:, 0:4:2])
                    for c in range(2):
                        ob = obp.tile([128, D], F32, name="ob", tag="ob")
                        for dh in range(2):
                            nc.vector.tensor_scalar_mul(
                                ob[:, dh * 512:(dh + 1) * 512],
                                ctx[(c, dh)][:, :],
                                rc[:, c:c + 1],
                            )
                            # fire each 512-col half as soon as it's scaled
                            e = nc.sync if dh == 0 else nc.scalar
                            e.dma_start(
                                o_d[s * QW + c * 128: s * QW + (c + 1) * 128,
                                    dh * 512:(dh + 1) * 512],
                                ob[:, dh * 512:(dh + 1) * 512],
                            )

            pending = deque()
            DEPTH = 2
            emitted = 0
            for s in range(NSLOT):
                nk = 4 * s + 4
                # vv[0..nk-1] writes must be emitted before this slot's ctx
                # matmuls reference them (Tile deps follow emission order).
                need = 2 * nk
                if emitted < need:
                    emit_v(need - emitted)
                    emitted = need
                while pending:
                    consume(pending.popleft())
                ctx_cur = {}
                for c in range(2):
                    for dh in range(2):
                        t = ctxp.tile(
                            [128, 512], F32,
                            name=f"ctx{c}{dh}", tag=f"ctx{c}{dh}",
                        )
                        ctx_cur[(c, dh)] = t
                dn_cur = dnp.tile([128, 4], F32, name="dn", tag="dn")
                for kb in range(nk):
                    ps_sc = rot.tile([128, 512], F32, name="rps", tag="rps")
                    for ec in range(NE):
                        nc.tensor.matmul(
                            ps_sc[:, 0:QW],
                            kx[ec][:, kb * 128:(kb + 1) * 128],
                            qT[ec][:, s * QW:(s + 1) * QW],
                            start=(ec == 0), stop=(ec == NE - 1),
                        )
                    t_idx = kb - (nk - 4)
                    if t_idx >= 0:
                        nc.vector.tensor_add(
                            ps_sc[:, 0:QW], ps_sc[:, 0:QW],
                            mask_t[:, t_idx * QW:(t_idx + 1) * QW],
                        )
                    pt = ptp.tile([128, QW], BF16, name="pt", tag="pt")
                    nc.scalar.activation(pt[:, :], ps_sc[:, 0:QW], EXP, scale=SCALE)
                    pending.append((s, kb, pt, ctx_cur, dn_cur))
                    if len(pending) > DEPTH:
                        consume(pending.popleft())
                    # weave one V chain between score blocks while any remain
                    if vq:
                        emit_v(1)
                        emitted += 1
            while pending:
                consume(pending.popleft())

    nc.compile()
    return nc


def _get_nc():
    if "nc" not in _cached:
        _cached["nc"] = _build()
    return _cached["nc"]


def build_in_maps(x, W_q, W_k, W_v):
    import ml_dtypes

    BF = ml_dtypes.bfloat16
    x = np.asarray(x, dtype=np.float32)
    wq = np.asarray(W_q, dtype=np.float32)
    wk = np.asarray(W_k, dtype=np.float32)
    m = np.ascontiguousarray(wq @ wk.T).astype(BF)
    wv = np.ascontiguousarray(np.asarray(W_v, dtype=np.float32).astype(BF))
    ones = np.ones((128, 2), dtype=BF)

    p = np.arange(128, dtype=np.int64)[:, None]
    f = np.arange(QW, dtype=np.int64)[None, :]
    masks_h = []
    for h in range(2):
        tiles = [
            np.where(128 * t + p <= 2 * f + h, np.float32(0.0), np.float32(MASK_NEG))
            for t in range(4)
        ]
        masks_h.append(np.concatenate(tiles, axis=1).astype(np.float32))

    xbf = x.astype(BF)
    xt_b = [np.ascontiguousarray(xbf[b].T) for b in range(B)]
    in_maps = []
    for c in range(8):
        b, h = c // 2, c % 2
        xq = np.ascontiguousarray(xbf[b, h::2, :].T)
        in_maps.append({
            "xt": xt_b[b],
            "xq": xq,
            "m": m,
            "wv": wv,
            "masks": masks_h[h],
            "ones": ones,
        })
    return in_maps


def kernel(x, W_q, W_k, W_v):
    from concourse.bass_utils import run_bass_kernel_spmd

    in_maps = build_in_maps(x, W_q, W_k, W_v)
    nc = _get_nc()
    res = run_bass_kernel_spmd(nc, in_maps, core_ids=list(range(8)))

    out = np.empty((B, S, D), dtype=np.float32)
    for c in range(8):
        b, h = c // 2, c % 2
        out[b, h::2, :] = res.results[c]["o"]
    return out
